# revision 7
# baseline (speedup 1.0000x reference)
"""BitLinear (BitNet b1.58) Trainium2 Bass kernel, token-sharded across 8 cores.

y = (round(clip(x/a_scale*127)) @ clip(round(W/w_scale),-1,1)^T) * w_scale*a_scale/127
  w_scale = mean(|W|)+eps (scalar), a_scale = max|x| per token + eps.

Strategy (per core, SPMD x8):
- x is sharded by tokens (16384/8 = 2048 per core); W replicated.
- Quantized activations (ints in [-127,127]) and ternary weights are exact in
  bf16; fp32 PSUM accumulation of <=2048 products (each |.|<=127) is exact, so
  the bf16 matmul is bit-exact integer arithmetic.
- Per token block [128, D]: abs-max reduce (DVE), round via the +-2^23 fp32
  trick (ACT+DVE), bf16 cast, SBUF->SBUF DMA-transpose into [D/128, 128, 128]
  i-major chunks, PE matmul vs pre-transposed ternary weights, scaled PSUM
  drain (ACT) with per-token output scale, DMA out.
- W: pass 1 reduces sum|W| (-> w_scale), pass 2 re-streams W, quantizes
  (clamp to +-1.4999999 then round trick) and DMA-transposes into a resident
  bf16 [128, D/128, O] rhs tensor.
"""

import sys

sys.path.insert(0, "/opt/trn_rl_repo")

import numpy as np

import concourse.bacc as bacc
import concourse.bass as bass
import concourse.bass_isa as bass_isa
import concourse.mybir as mybir
import concourse.tile as tile

P = 128
MM_N = 512  # free-dim per matmul (one PSUM bank)
EXP23 = 12582912.0  # 1.5*2**23; v + C - C == RNE round for |v| < 2**22
EPS = 1e-8
CLIP_HI = 1.4999999  # largest f32 < 1.5; clamp-then-round == clip(round(.),-1,1)
N_CORES = 8
F32 = mybir.dt.float32
BF16 = mybir.dt.bfloat16
ALU = mybir.AluOpType
AFT = mybir.ActivationFunctionType
AX = mybir.AxisListType


def emit_bitlinear(
    tc: "tile.TileContext",
    y: "bass.AP",
    xs: "bass.AP",
    w: "bass.AP",
    repeat: int = 1,
):
    nc = tc.nc
    T, D = xs.shape
    O = w.shape[0]
    TB, NI, NR = T // P, D // P, O // P
    NOB = O // MM_N

    from contextlib import ExitStack

    with ExitStack() as ctx:
        if repeat > 1:
            ctx.enter_context(tc.For_i(0, repeat, 1))
        wload = ctx.enter_context(tc.tile_pool(name="wload", bufs=3))
        small = ctx.enter_context(tc.tile_pool(name="small", bufs=1))
        wq_pool = ctx.enter_context(tc.tile_pool(name="wq", bufs=2))
        wqt_pool = ctx.enter_context(tc.tile_pool(name="wqt", bufs=1))
        xpool = ctx.enter_context(tc.tile_pool(name="xp", bufs=3))
        tpool = ctx.enter_context(tc.tile_pool(name="tp", bufs=2))
        aq_pool = ctx.enter_context(tc.tile_pool(name="aqp", bufs=2))
        aqt_pool = ctx.enter_context(tc.tile_pool(name="aqtp", bufs=3))
        ypool = ctx.enter_context(tc.tile_pool(name="yp", bufs=2))
        sc_pool = ctx.enter_context(tc.tile_pool(name="scp", bufs=4))
        ps_pool = ctx.enter_context(tc.tile_pool(name="psp", bufs=2, space="PSUM"))

        # ---------------- W pass 1: w_scale = mean(|W|) + eps ----------------
        wsums = small.tile([P, NR], F32)
        for r in range(NR):
            wt = wload.tile([P, D], F32, tag="wt", name=f"wt_{r}")
            nc.sync.dma_start(wt[:], w[r * P : (r + 1) * P, :])
            nc.vector.tensor_reduce(
                wsums[:, r : r + 1], wt[:], axis=AX.X, op=ALU.add,
                apply_absolute_value=True,
            )
        wsum1 = small.tile([P, 1], F32)
        nc.vector.tensor_reduce(wsum1[:], wsums[:], axis=AX.X, op=ALU.add)
        wsum_all = small.tile([P, 1], F32)
        nc.gpsimd.partition_all_reduce(
            wsum_all[:], wsum1[:], channels=P, reduce_op=bass_isa.ReduceOp.add
        )
        w_scale = small.tile([P, 1], F32)
        nc.vector.tensor_scalar(
            w_scale[:], wsum_all[:], 1.0 / (O * D), EPS, op0=ALU.mult, op1=ALU.add
        )
        inv_w = small.tile([P, 1], F32)
        nc.vector.reciprocal(inv_w[:], w_scale[:])

        # ------- W pass 2: quantize to ternary bf16, transpose into wqT -------
        # wqT[p, j, o] = Wq[o, j*128 + p]
        wqT = wqt_pool.tile([P, NI, O], BF16)
        for r in range(NR):
            wt2 = wload.tile([P, D], F32, tag="wt", name=f"wt2_{r}")
            nc.sync.dma_start(wt2[:], w[r * P : (r + 1) * P, :])
            u = wq_pool.tile([P, D], F32, tag="u", name=f"u_{r}")
            nc.vector.tensor_scalar(
                u[:], wt2[:], inv_w[:], CLIP_HI, op0=ALU.mult, op1=ALU.min
            )
            v = wq_pool.tile([P, D], F32, tag="v", name=f"v_{r}")
            nc.vector.tensor_scalar(
                v[:], u[:], -CLIP_HI, EXP23, op0=ALU.max, op1=ALU.add
            )
            wqr = wq_pool.tile([P, D], BF16, tag="wqr", name=f"wqr_{r}")
            nc.scalar.activation(wqr[:], v[:], AFT.Copy, bias=-EXP23)
            nc.sync.dma_start(
                wqT[:, :, r * P : (r + 1) * P], wqr[:], transpose=True
            )

        # ---------------- per-token-block pipeline ----------------
        for tb in range(TB):
            xt = xpool.tile([P, D], F32, tag="x", name=f"x_{tb}")
            nc.sync.dma_start(xt[:], xs[tb * P : (tb + 1) * P, :])

            amax = sc_pool.tile([P, 1], F32, tag="amax", name=f"amax_{tb}")
            nc.vector.tensor_reduce(
                amax[:], xt[:], axis=AX.X, op=ALU.max, apply_absolute_value=True
            )
            a_eps = sc_pool.tile([P, 1], F32, tag="aeps", name=f"aeps_{tb}")
            nc.vector.tensor_scalar_add(a_eps[:], amax[:], EPS)
            rcp = sc_pool.tile([P, 1], F32, tag="rcp", name=f"rcp_{tb}")
            nc.vector.reciprocal(rcp[:], a_eps[:])
            inv127 = sc_pool.tile([P, 1], F32, tag="i127", name=f"i127_{tb}")
            nc.vector.tensor_scalar_mul(inv127[:], rcp[:], 127.0)
            # output scale: a_scale * w_scale / 127
            es = sc_pool.tile([P, 1], F32, tag="es", name=f"es_{tb}")
            nc.vector.tensor_scalar(
                es[:], a_eps[:], w_scale[:], 1.0 / 127.0, op0=ALU.mult, op1=ALU.mult
            )

            # round(x * inv127): ACT does x*inv127 + 2^23, DVE subtracts -> bf16
            t = tpool.tile([P, D], F32, tag="t", name=f"t_{tb}")
            nc.scalar.activation(
                t[:], xt[:], AFT.Copy, bias=EXP23, scale=inv127[:]
            )
            aq = aq_pool.tile([P, D], BF16, tag="aq", name=f"aq_{tb}")
            nc.vector.tensor_scalar_add(aq[:], t[:], -EXP23)

            # aqT[p, j, t] = aq[t, j*128 + p]
            aqT = aqt_pool.tile([P, NI, P], BF16, tag="aqT", name=f"aqT_{tb}")
            nc.sync.dma_start(aqT[:], aq[:], transpose=True)

            ps = ps_pool.tile([P, O], F32, tag="ps", name=f"ps_{tb}")
            for j in range(NI):
                for ob in range(NOB):
                    nc.tensor.matmul(
                        ps[:, ob * MM_N : (ob + 1) * MM_N],
                        lhsT=aqT[:, j, :],
                        rhs=wqT[:, j, ob * MM_N : (ob + 1) * MM_N],
                        start=(j == 0),
                        stop=(j == NI - 1),
                    )

            ysb = ypool.tile([P, O], F32, tag="y", name=f"y_{tb}")
            nc.scalar.activation(ysb[:], ps[:], AFT.Copy, scale=es[:])
            nc.sync.dma_start(y[tb * P : (tb + 1) * P, :], ysb[:])


_NC_CACHE: dict = {}


def _get_nc(T: int, D: int, O: int, repeat: int = 1) -> "bass.Bass":
    key = (T, D, O, repeat)
    if key not in _NC_CACHE:
        nc = bacc.Bacc("TRN2", target_bir_lowering=False, debug=False)
        xs = nc.dram_tensor("xs", [T, D], F32, kind="ExternalInput").ap()
        w = nc.dram_tensor("w", [O, D], F32, kind="ExternalInput").ap()
        y = nc.dram_tensor("y", [T, O], F32, kind="ExternalOutput").ap()
        with tile.TileContext(nc) as tc:
            emit_bitlinear(tc, y, xs, w, repeat=repeat)
        nc.compile()
        _NC_CACHE[key] = nc
    return _NC_CACHE[key]


def kernel(
    x: np.ndarray, weight: np.ndarray, _trace: bool = False, _repeat: int = 1
):
    from concourse.bass_utils import run_bass_kernel_spmd

    x = np.asarray(x, dtype=np.float32)
    weight = np.ascontiguousarray(np.asarray(weight, dtype=np.float32))
    B, S, D = x.shape
    O = weight.shape[0]
    tokens = B * S
    Tc = tokens // N_CORES
    xf = np.ascontiguousarray(x.reshape(tokens, D))

    nc = _get_nc(Tc, D, O, repeat=_repeat)
    in_maps = [
        {"xs": np.ascontiguousarray(xf[c * Tc : (c + 1) * Tc]), "w": weight}
        for c in range(N_CORES)
    ]
    res = run_bass_kernel_spmd(
        nc, in_maps, list(range(N_CORES)), trace=_trace
    )
    out = np.concatenate([res.results[c]["y"] for c in range(N_CORES)], axis=0)
    out = out.reshape(B, S, O)
    if _trace:
        return out, res
    return out
